# revision 25
# baseline (speedup 1.0000x reference)
"""Distributed Trainium2 kernel for nn_Attention (B=2, N=4096, C=512, H=8).

Sharding: 8 cores = (batch in {0,1}) x (head-pair in {0..3}).
Each core computes the QKV projection for its 2 heads, full NxN attention
for those heads, and the partial output projection (its 2 heads' rows of
w_proj), emitted as bf16.  The host sums the 4 partials per batch in f32
and adds b_proj.

Device dataflow (all-transposed, zero on-device attention transposes):
  - host passes x[b].T and the weight slices pre-cast to bf16, so
    activations arrive contraction-major (no on-device transposes)
  - qT/kT [64d x N] per head (head-dim on partitions); V in natural
    [k x d] layout augmented with a ones column, so the softmax
    denominator falls out of the A.V matmul for free
  - scores are computed transposed, sT[k,q] = K @ qT, with the two heads
    row-packed on the PE (two concurrent K=64 matmuls); the exp'd tile is
    then exactly the stationary operand the A.V matmul needs
  - softmax exp alternates between ScalarE (exact, table-based) and
    VectorE (Schraudolph-style exp: bits16 = A*s + B, f32->int16 convert,
    bitcast to bf16; ~1.8% rms washes out post-normalization)
  - AV: outT[d(+1),q] += V_aug.T @ attn, accumulated over 32 k-chunks in
    PSUM; the av matmul for chunk kc trails the exp by 2 chunks so exp
    latency never stalls the PE
  - schedule: qtile 0 runs SOLO with the 8 QKV prologues interleaved into
    its k-sweep (solo sweeps hold only 2 of the 4 "av" PSUM slots, so the
    prologue q/k/v matmuls borrow the free banks and the x DMA overlaps
    attention instead of serializing in front of it); then qtile pairs
    (1,2),(3,4),(5,6); qtile 7 runs solo at the end to shorten the
    exposed post-processing tail
  - per-qtile post (pipelined into the following sweeps): ScalarE copies
    AV out of PSUM, VectorE reciprocal of the denominator row, GPSIMD
    partition_broadcast + multiply for the normalization, single K=128
    projection matmul per 128 rows, one batched bf16 output DMA per qtile
"""

import numpy as np
import ml_dtypes

B, N, C = 2, 4096, 512
H = 8
HD = C // H           # 64
SCALE = HD ** -0.5
NCORES = 8
RG = N // 512         # 8 row groups of 512
QT = N // 512         # 8 q tiles of 512
KC = N // 128         # 32 k chunks of 128
PEND = 6              # av matmul trails exp by this many k-chunks
POPS_PAIR = (4, 10, 18)      # stage_q pop points within a pair k-sweep
POPS_SOLO = (4, 10, 18)      # stage_q pop points within a solo k-sweep

_CACHE = {}


def _pbcast(ap, nparts):
    """AP view that reads partition 0 of `ap` broadcast over nparts partitions."""
    import concourse.bass as bass
    return bass.AP(tensor=ap.tensor, offset=ap.offset, ap=[[0, nparts]] + list(ap.ap[1:]))


def _build_bass(loop_reps=None):
    import concourse.bass as bass
    import concourse.mybir as mybir
    import concourse.tile as tile
    from concourse import bacc

    f32 = mybir.dt.float32
    bf16 = mybir.dt.bfloat16
    i16 = mybir.dt.int16
    Exp = mybir.ActivationFunctionType.Exp
    # Schraudolph exp -> bf16 bits on DVE: bits16 = A*s + B (f32->int16
    # convert rounds to nearest), bitcast to bf16.  Offloads ~1/2 of the
    # softmax exp work from the ScalarE to the DVE.
    SCHR_A = float(128.0 * np.log2(np.e) * SCALE)
    SCHR_B = float(127.0 * 128.0 - 7.4)

    nc = bacc.Bacc()

    xt_d = nc.dram_tensor("xt", [C, N], bf16, kind="ExternalInput")
    w_d = nc.dram_tensor("w", [C, 3 * 2 * HD], bf16, kind="ExternalInput")
    wp_d = nc.dram_tensor("wp", [2 * HD, C], bf16, kind="ExternalInput")
    out_d = nc.dram_tensor("out", [N, C], bf16, kind="ExternalOutput")
    # last qtile is normalized on the host: per-head partial projections +
    # softmax denominators (shortens the exposed device-side tail)
    p7_d = nc.dram_tensor("p7", [2, 512, C], bf16, kind="ExternalOutput")
    dn7_d = nc.dram_tensor("dn7", [2, 512], bf16, kind="ExternalOutput")

    with tile.TileContext(nc) as tc:
        from concourse import library_config
        nc.gpsimd.load_library(library_config.proxy)
        with (
            tc.tile_pool(name="persist", bufs=1) as persist,
            tc.tile_pool(name="attn", bufs=10) as apool,
            tc.tile_pool(name="norm", bufs=2) as npool,
            tc.tile_pool(name="small", bufs=4) as small,
            tc.tile_pool(name="y", bufs=8) as ypool,
            tc.tile_pool(name="spsum", bufs=2, space="PSUM") as spool,
            tc.tile_pool(name="avpsum", bufs=4, space="PSUM") as avpool,
        ):
            # ---- weights (per-ctr chunks, interleaved with x0 at emit) ----
            w_sb = persist.tile([128, 4, 6 * HD], bf16, tag="w")
            wp_sb = persist.tile([128, C], bf16, tag="wp")
            # per-head wp rows re-based to partition 0 (for the host-
            # normalized last qtile's per-head projections)
            wp2_sb = persist.tile([HD, 2, C], bf16, tag="wp2")

            def emit_w_dma():
                nc.sync.dma_start(
                    w_sb[:, :, :],
                    w_d[:, :].rearrange("(ctr p) n -> p ctr n", ctr=4),
                )

            # ---- persistent per-row-group tiles ----
            xsb = [persist.tile([128, 4, 512], bf16, tag=f"xsb{rg}", name=f"xsb{rg}") for rg in range(RG)]
            qTt = [persist.tile([128, 512], bf16, tag=f"qT{rg}", name=f"qT{rg}") for rg in range(RG)]
            kTt = [persist.tile([128, 512], bf16, tag=f"kT{rg}", name=f"kT{rg}") for rg in range(RG)]
            vt = [persist.tile([128, 2, 4, HD + 1], bf16, tag=f"v{rg}", name=f"v{rg}") for rg in range(RG)]

            def emit_x_dma(rg, split):
                src = xt_d[:, rg * 512:(rg + 1) * 512]
                if split:
                    for ctr in range(4):
                        nc.sync.dma_start(
                            xsb[rg][:, ctr, :],
                            xt_d[ctr * 128:(ctr + 1) * 128, rg * 512:(rg + 1) * 512],
                        )
                else:
                    nc.sync.dma_start(
                        xsb[rg][:, :, :],
                        src.rearrange("(ctr p) n -> p ctr n", ctr=4),
                    )

            def prologue(rg):
                # qT/kT: [c_out(2 heads x 64) x rows] = w.T @ x.T-chunk
                # (q in plane 0, k in plane 1 of one scores-ring psum tile)
                ps_qk = spool.tile([128, 2, 512], f32, tag="scores", name=f"psqk{rg}")
                for ctr in range(4):
                    nc.tensor.matmul(
                        ps_qk[:, 0, :], w_sb[:, ctr, 0:2 * HD], xsb[rg][:, ctr, :],
                        start=(ctr == 0), stop=(ctr == 3),
                    )
                for ctr in range(4):
                    nc.tensor.matmul(
                        ps_qk[:, 1, :], w_sb[:, ctr, 2 * HD:4 * HD], xsb[rg][:, ctr, :],
                        start=(ctr == 0), stop=(ctr == 3),
                    )
                nc.scalar.copy(qTt[rg][:, :], ps_qk[:, 0, :])
                nc.scalar.copy(kTt[rg][:, :], ps_qk[:, 1, :])
                # V natural layout: [rows x c_out]; rows on partitions
                ps_v = avpool.tile([128, 4, 128], f32, tag="av", name=f"psv{rg}")
                for rcl in range(4):
                    for ctr in range(4):
                        nc.tensor.matmul(
                            ps_v[:, rcl, :],
                            xsb[rg][:, ctr, rcl * 128:(rcl + 1) * 128],
                            w_sb[:, ctr, 4 * HD:6 * HD],
                            start=(ctr == 0), stop=(ctr == 3),
                        )
                # scatter into v tiles on the DVE: [128k x (h, rcl, d)] ; ones col
                nc.vector.tensor_scalar_mul(
                    vt[rg][:, :, :, 0:HD],
                    ps_v.rearrange("p rcl (h d) -> p h rcl d", h=2),
                    1.0,
                )
                nc.vector.memset(vt[rg][:, :, :, HD:HD + 1], 1.0)

            def qk_exp(qt, kc, use_act):
                rgk, kcl = divmod(kc, 4)
                sco = spool.tile([128, 2, 512], f32, tag="scores")
                # scores.T [k x q], two heads row-packed (K=64 each)
                nc.tensor.matmul(
                    sco[:, 0, :],
                    kTt[rgk][0:HD, kcl * 128:(kcl + 1) * 128],
                    qTt[qt][0:HD, :],
                    start=True, stop=True, skip_group_check=True,
                )
                nc.tensor.matmul(
                    sco[:, 1, :],
                    kTt[rgk][HD:2 * HD, kcl * 128:(kcl + 1) * 128],
                    qTt[qt][HD:2 * HD, :],
                    start=True, stop=True, skip_group_check=True,
                )
                att = apool.tile([128, 2, 512], bf16, tag="attn")
                if use_act:
                    nc.scalar.activation(att[:, :, :], sco[:, :, :], Exp, scale=SCALE)
                else:
                    nc.vector.tensor_scalar(
                        att.bitcast(i16)[:, :, :], sco[:, :, :], SCHR_A, SCHR_B,
                        op0=mybir.AluOpType.mult, op1=mybir.AluOpType.add,
                    )
                return att

            def av_acc(qt, kc, att, av0, av1, first, last):
                rgk, kcl = divmod(kc, 4)
                nc.tensor.matmul(
                    av0[:, :], vt[rgk][:, 0, kcl, :], att[:, 0, :],
                    start=first, stop=last, skip_group_check=True,
                )
                nc.tensor.matmul(
                    av1[:, :], vt[rgk][:, 1, kcl, :], att[:, 1, :],
                    start=first, stop=last, skip_group_check=True,
                )

            def post_stage1(qt, av0, av1, on_dve=False):
                # move AV out of PSUM (frees the av psum slots); pair units
                # split the copies across ScalarE and VectorE so the boundary
                # handoff isn't gated on one engine's backlog
                avsb = npool.tile([128, 2, 512], f32, tag="avsb", name=f"avsb{qt}")
                if on_dve:
                    nc.vector.tensor_scalar_mul(avsb[0:HD + 1, 0, :], av0[:, :], 1.0)
                    nc.vector.tensor_scalar_mul(avsb[0:HD + 1, 1, :], av1[:, :], 1.0)
                else:
                    nc.scalar.copy(avsb[0:HD + 1, 0, :], av0[:, :])
                    nc.scalar.copy(avsb[0:HD + 1, 1, :], av1[:, :])
                return avsb

            def post_stage1b(qt, avsb):
                rec0 = small.tile([1, 512], f32, tag="rec0", name=f"rec0_{qt}")
                rec1 = small.tile([1, 512], f32, tag="rec1", name=f"rec1_{qt}")
                nc.vector.reciprocal(rec0[:, :], avsb[HD:HD + 1, 0, :])
                nc.vector.reciprocal(rec1[:, :], avsb[HD:HD + 1, 1, :])
                return avsb, rec0, rec1

            def post_stage2(qt, avsb, rec0, rec1):
                rbc0 = small.tile([HD, 512], f32, tag="rbc0", name=f"rbc0_{qt}")
                rbc1 = small.tile([HD, 512], f32, tag="rbc1", name=f"rbc1_{qt}")
                nc.gpsimd.partition_broadcast(rbc0[:, :], rec0[:, :])
                nc.gpsimd.partition_broadcast(rbc1[:, :], rec1[:, :])
                # normalized attention output, both heads stacked [128 x 512q]
                avn = npool.tile([128, 512], bf16, tag="avn", name=f"avn{qt}")
                nc.gpsimd.tensor_mul(avn[0:HD, :], avsb[0:HD, 0, :], rbc0[:, :])
                nc.gpsimd.tensor_mul(avn[HD:2 * HD, :], avsb[0:HD, 1, :], rbc1[:, :])
                return avn

            def post_stage3(qt, avn):
                for qc in range(4):
                    ps_y = avpool.tile([128, 512], f32, tag="av", name=f"psy{qt}_{qc}")
                    nc.tensor.matmul(
                        ps_y[:, :], avn[:, qc * 128:(qc + 1) * 128],
                        wp_sb[:, :], start=True, stop=True,
                        skip_group_check=True,
                    )
                    y_sb = ypool.tile([128, 512], bf16, tag="y", name=f"y{qt}_{qc}")
                    nc.scalar.copy(y_sb[:, :], ps_y[:, :])
                    nc.sync.dma_start(
                        out_d[qt * 512 + qc * 128: qt * 512 + (qc + 1) * 128, :],
                        y_sb[:, :],
                    )

            # ---- emission ----
            def emit_body():
                avs = {}
                pend = {}
                stage_q = []

                def push(qt, kc, use_act):
                    att = qk_exp(qt, kc, use_act)
                    pend.setdefault(qt, []).append((kc, att))
                    if len(pend[qt]) > PEND:
                        pkc, patt = pend[qt].pop(0)
                        av_acc(qt, pkc, patt, *avs[qt], pkc == 0, pkc == KC - 1)

                def drain_av(qt):
                    for pkc, patt in pend.pop(qt):
                        av_acc(qt, pkc, patt, *avs[qt], pkc == 0, pkc == KC - 1)

                def mk1b_pair(qa, sa, qb, sb_):
                    def s1b():
                        ca = post_stage1b(qa, sa)
                        cb = post_stage1b(qb, sb_)

                        def s2():
                            avn_a = post_stage2(qa, *ca)
                            avn_b = post_stage2(qb, *cb)
                            stage_q.append(lambda: (post_stage3(qa, avn_a),
                                                    post_stage3(qb, avn_b)))
                        stage_q.insert(0, s2)
                    return s1b

                def mk1b_solo(qt, sb_):
                    def s1b():
                        c = post_stage1b(qt, sb_)

                        def s2():
                            avn = post_stage2(qt, *c)
                            stage_q.append(lambda: post_stage3(qt, avn))
                        stage_q.insert(0, s2)
                    return s1b

                def new_avs(qt):
                    avs[qt] = (
                        avpool.tile([HD + 1, 512], f32, tag="av", name=f"av0_t{qt}"),
                        avpool.tile([HD + 1, 512], f32, tag="av", name=f"av1_t{qt}"),
                    )

                # DMAs up front; w chunks interleaved with x0 so the first
                # prologue matmul starts after one w-chunk + one x-chunk
                emit_w_dma()
                emit_x_dma(0, split=True)
                emit_x_dma(1, split=True)
                nc.sync.dma_start(wp_sb[:, :], wp_d[:, :])
                for h in range(2):
                    nc.sync.dma_start(wp2_sb[:, h, :], wp_d[h * HD:(h + 1) * HD, :])
                for rg in range(2, RG):
                    emit_x_dma(rg, split=False)

                # finalize closure for the previous sweep unit: runs after
                # the next unit's first 2 pushes, so drain/copies overlap the
                # new unit's score matmuls instead of stalling the PE
                def mk_finalize(qts):
                    def fin():
                        sbs = []
                        for i, qt in enumerate(qts):
                            drain_av(qt)
                            sbs.append(post_stage1(qt, *avs.pop(qt), on_dve=(i == 1)))
                        if stage_q:
                            stage_q.pop(0)()
                        if len(qts) == 2:
                            stage_q.append(mk1b_pair(qts[0], sbs[0], qts[1], sbs[1]))
                        else:
                            stage_q.append(mk1b_solo(qts[0], sbs[0]))
                    return fin

                def finalize_qt7():
                    # host-normalized tail: skip the on-device softmax divide;
                    # emit per-head unnormalized projections + denominators
                    drain_av(7)
                    av0, av1 = avs.pop(7)
                    # both heads' AV staged at partition base 0 (no cross-
                    # partition engine moves); row HD carries the denominators
                    avn7 = npool.tile([HD + 1, 2, 512], bf16, tag="avn7", name="avn7")
                    nc.scalar.copy(avn7[:, 0, :], av0[:, :])
                    nc.vector.tensor_scalar_mul(avn7[:, 1, :], av1[:, :], 1.0)
                    nc.sync.dma_start(dn7_d[:, :], avn7[HD:HD + 1, :, :])
                    y7 = [ypool.tile([128, 4, 512], bf16, tag="y7", bufs=2,
                                     name=f"y7_{h}") for h in range(2)]
                    for qc in range(4):
                        for h in range(2):
                            ps = avpool.tile([128, 512], f32, tag="av", name=f"psy7_{qc}_{h}")
                            nc.tensor.matmul(
                                ps[:, :], avn7[0:HD, h, qc * 128:(qc + 1) * 128],
                                wp2_sb[:, h, :],
                                start=True, stop=True, skip_group_check=True,
                            )
                            if h == 0:
                                nc.scalar.copy(y7[h][:, qc, :], ps[:, :])
                            else:
                                nc.vector.tensor_scalar_mul(y7[h][:, qc, :], ps[:, :], 1.0)
                    for h in range(2):
                        nc.sync.dma_start(
                            p7_d[h, :, :].rearrange("(qc p) n -> p qc n", qc=4),
                            y7[h][:, :, :],
                        )

                prologue(0)
                prologue(1)

                units = [(0,), (1, 2), (3, 4), (5, 6), (7,)]
                fin_prev = None
                for ui, qts in enumerate(units):
                    for qt in qts:
                        new_avs(qt)
                    solo = len(qts) == 1
                    last = ui == len(units) - 1
                    for kc in range(KC):
                        if solo:
                            # last unit: put the final exps on the faster
                            # ScalarE so the tail drain isn't DVE-gated
                            push(qts[0], kc, use_act=(kc % 2 == (1 if last else 0)))
                        else:
                            push(qts[0], kc, True)
                            push(qts[1], kc, False)
                        if kc == 1 and fin_prev:
                            fin_prev()
                            fin_prev = None
                        if ui == 0 and kc % 4 == 2 and kc < 24:
                            prologue(kc // 4 + 2)
                        popk = POPS_SOLO if solo else POPS_PAIR
                        if kc in popk and stage_q:
                            stage_q.pop(0)()
                    fin_prev = mk_finalize(qts) if not last else finalize_qt7
                fin_prev()
                while stage_q:
                    stage_q.pop(0)()

            if loop_reps:
                with tc.For_i(0, loop_reps, 1):
                    emit_body()
            else:
                emit_body()

    nc.compile()
    return nc


def _get_nc():
    if "nc" not in _CACHE:
        _CACHE["nc"] = _build_bass()
    return _CACHE["nc"]


def _make_in_maps(x, w_qkv, w_proj):
    bf = ml_dtypes.bfloat16
    in_maps = []
    for core in range(NCORES):
        b, j = divmod(core, 4)
        xt = np.ascontiguousarray(x[b].T).astype(bf)            # [C, N]
        w = np.ascontiguousarray(np.concatenate([
            w_qkv[:, 128 * j:128 * j + 128],
            w_qkv[:, C + 128 * j:C + 128 * j + 128],
            w_qkv[:, 2 * C + 128 * j:2 * C + 128 * j + 128],
        ], axis=1)).astype(bf)                                  # [C, 384]
        wp = np.ascontiguousarray(w_proj[128 * j:128 * j + 128, :]).astype(bf)
        in_maps.append({"xt": xt, "w": w, "wp": wp})
    return in_maps


def _run(x, w_qkv, w_proj, b_proj, trace=False):
    from concourse.bass_utils import run_bass_kernel_spmd

    nc = _get_nc()
    in_maps = _make_in_maps(x, w_qkv, w_proj)
    res = run_bass_kernel_spmd(nc, in_maps, core_ids=list(range(NCORES)), trace=trace)
    out = np.zeros((B, N, C), dtype=np.float32)
    q7 = N - 512
    for core in range(NCORES):
        b = core // 4
        r = res.results[core]
        out[b, :q7] += np.asarray(r["out"], dtype=np.float32)[:q7]
        p7 = r["p7"].astype(np.float32)          # [2, 512, C] per-head partials
        dn7 = np.asarray(r["dn7"], dtype=np.float32)  # [2, 512]
        out[b, q7:] += p7[0] / dn7[0][:, None] + p7[1] / dn7[1][:, None]
    out += b_proj.astype(np.float32)
    return out, res


def kernel(x, w_qkv, w_proj, b_proj):
    x = np.asarray(x, dtype=np.float32)
    w_qkv = np.asarray(w_qkv, dtype=np.float32)
    w_proj = np.asarray(w_proj, dtype=np.float32)
    b_proj = np.asarray(b_proj, dtype=np.float32)
    out, _ = _run(x, w_qkv, w_proj, b_proj, trace=False)
    return out
